# revision 2
# baseline (speedup 1.0000x reference)
"""HaarDeconv2D (vertical, 2x1, stride (2,1)) Trainium2 kernel.

Math: with L=[0.5,0.5], D=[0.5,-0.5],
  even = 0.5*(low+detail) + 0.5*(low-detail) = low_pass
  odd  = 0.5*(low+detail) - 0.5*(low-detail) = detail
so the output is exactly a row-interleave of the two inputs along H.
This is pure data movement: two strided DRAM->DRAM DMAs per core, no
compute engines at all. Sharded over batch (16 -> 2 per core, 8 cores).
"""

import numpy as np

_N_CORES = 8
_B, _C, _H, _W = 16, 3, 512, 1024
_BS = _B // _N_CORES  # batches per core

_nc_cache = None


def _build():
    global _nc_cache
    if _nc_cache is not None:
        return _nc_cache
    import concourse.bass as bass
    import concourse.mybir as mybir

    nc = bass.Bass()
    low = nc.dram_tensor(
        "low", [_BS, _C, _H, _W], mybir.dt.float32, kind="ExternalInput"
    )
    det = nc.dram_tensor(
        "det", [_BS, _C, _H, _W], mybir.dt.float32, kind="ExternalInput"
    )
    out = nc.dram_tensor(
        "out", [_BS, _C, 2 * _H, _W], mybir.dt.float32, kind="ExternalOutput"
    )
    # out viewed as [b, c, h, 2, w]: slot 0 rows come from low, slot 1 from det
    ov = out[:].rearrange("b c (h two) w -> b c h two w", two=2)

    with nc.Block() as block, nc.semaphore("dma_sem") as dma_sem:
        # Two HWDGE rings (qSyncDynamicHW + qScalarDynamicHW): the 16 SDMA
        # engines round-robin packets across both queues, doubling the
        # outstanding requests per engine vs a single queue.
        @block.sync
        def _(sync):
            sync.dma_start(out=ov[:, :, :, 0, :], in_=low[:]).then_inc(dma_sem, 16)
            sync.wait_ge(dma_sem, 32)

        @block.scalar
        def _(scalar):
            scalar.dma_start(out=ov[:, :, :, 1, :], in_=det[:]).then_inc(dma_sem, 16)
            scalar.wait_ge(dma_sem, 32)

    _nc_cache = nc
    return nc


def kernel(low_pass, detail):
    from concourse.bass_utils import run_bass_kernel_spmd

    low_pass = np.ascontiguousarray(np.asarray(low_pass, dtype=np.float32))
    detail = np.ascontiguousarray(np.asarray(detail, dtype=np.float32))
    nc = _build()
    in_maps = [
        {
            "low": low_pass[i * _BS : (i + 1) * _BS],
            "det": detail[i * _BS : (i + 1) * _BS],
        }
        for i in range(_N_CORES)
    ]
    r = run_bass_kernel_spmd(nc, in_maps, core_ids=list(range(_N_CORES)))
    return np.concatenate([res["out"] for res in r.results], axis=0)


# revision 3
# speedup vs baseline: 1.1007x; 1.1007x over previous
"""HaarDeconv2D (vertical, 2x1, stride (2,1)) Trainium2 kernel.

Math: with L=[0.5,0.5], D=[0.5,-0.5],
  even = 0.5*(low+detail) + 0.5*(low-detail) = low_pass
  odd  = 0.5*(low+detail) - 0.5*(low-detail) = detail
so the output is exactly a row-interleave of the two inputs along H.
This is pure data movement. Each core gets a [2, BS*C, H, W] stack of
(low, detail) shards and performs the interleave with a single strided
DRAM->DRAM DMA whose write stream is fully contiguous (sequential HBM
writes; reads advance two sequential cursors). Sharded over batch
(16 -> 2 per core, 8 cores).
"""

import numpy as np

_N_CORES = 8
_B, _C, _H, _W = 16, 3, 512, 1024
_BS = _B // _N_CORES  # batches per core
_M = _BS * _C  # merged batch*channel dim per core

_nc_cache = None


def _build():
    global _nc_cache
    if _nc_cache is not None:
        return _nc_cache
    import concourse.bass as bass
    import concourse.mybir as mybir

    nc = bass.Bass()
    inp = nc.dram_tensor(
        "inp", [2, _M, _H, _W], mybir.dt.float32, kind="ExternalInput"
    )
    out = nc.dram_tensor(
        "out", [_BS, _C, 2 * _H, _W], mybir.dt.float32, kind="ExternalOutput"
    )
    # src: read in (m, h, s, w) order so the destination is contiguous
    src = inp[:].rearrange("s m h w -> m h s w")
    dst = out[:].rearrange("b c (h s) w -> (b c) h s w", s=2)

    with nc.Block() as block, nc.semaphore("dma_sem") as dma_sem:

        @block.sync
        def _(sync):
            sync.dma_start(out=dst, in_=src).then_inc(dma_sem, 16)
            sync.wait_ge(dma_sem, 16)

    _nc_cache = nc
    return nc


def _shard_inputs(low_pass, detail):
    low_pass = np.asarray(low_pass, dtype=np.float32)
    detail = np.asarray(detail, dtype=np.float32)
    in_maps = []
    for i in range(_N_CORES):
        lo = low_pass[i * _BS : (i + 1) * _BS].reshape(_M, _H, _W)
        de = detail[i * _BS : (i + 1) * _BS].reshape(_M, _H, _W)
        in_maps.append({"inp": np.stack([lo, de])})
    return in_maps


def kernel(low_pass, detail):
    from concourse.bass_utils import run_bass_kernel_spmd

    nc = _build()
    in_maps = _shard_inputs(low_pass, detail)
    r = run_bass_kernel_spmd(nc, in_maps, core_ids=list(range(_N_CORES)))
    return np.concatenate([res["out"] for res in r.results], axis=0)
